# revision 33
# baseline (speedup 1.0000x reference)
"""Self-attention (IntraSelfAttention) kernel for Trainium2, 8-core data
parallel, fp8e4m3 DoubleRow matmuls.

Math (per batch element b, on one core), with E = exp(A @ A.T):
    out = (E @ Vm) / (E @ m + eps) * m_row,   Vm = A * m[:,None]

Decompose E = 1 + f, f = expm1(QK).  Then
    E @ Vm = colsum(Vm) + f @ Vm        (colsum added on host, rank-1)
    E @ m  = sum(m) + f @ m             (sum(m) DMA'd as a constant)
f has zero mean +- 0.07 off-diagonal, so it survives fp8e4m3 quantization
(x16) where E itself (~1.0, fp8 ulp 0.125) would not.  Both matmuls run
in fp8 DoubleRow mode (2 contraction rows/cycle):
    QK:  psum = sum_k at8[:,2k:2k+2,i] .T@ at8[:,2k:2k+2,j]   (= 4096*QK)
    exp: scalar activation Exp, scale=1/4096 -> E fp16 scratch
    f:   gpsimd (E - 1)*16 -> fp8 tiles Epair[q][p, u, s]
    AV:  pa = f8 @ v8[0:512], pb = f8 @ v8[512:769] (col 768 = 64*mask)
    out = pa * rinv (fp16), rinv = 1/(pb[:,256] + 1024*(summ+eps))
Host: out_full = (out + 1024*rinv * colsum(Vm)) * mask_row.

Symmetry of f means QK's output tiles are directly usable as AV lhsT.
Warm-up dummy matmuls run while the input DMA streams so the PE HAM
clock-gate is released (2.4 GHz) by the time real matmuls start.
"""

import os
import numpy as np
import ml_dtypes

try:
    import concourse.bass as bass
except ImportError:
    import sys

    sys.path.insert(0, "/opt/trn_rl_repo")
    import concourse.bass as bass

import concourse.mybir as mybir
import concourse.tile as tile
from concourse import bass_utils
from concourse.tile_sem_assignment import PROC_NAME_TO_IDX

_IDX2PROC = {v: k for k, v in PROC_NAME_TO_IDX.items()}


def _split_drain_and_barrier(self, tick_clock, wait_clock):
    """Replacement for TileContext._drain_and_barrier.

    The stock version attaches every outstanding semaphore wait to the single
    kernel-tail Drain instruction; walrus's per-instruction sync-wait capacity
    is tiny, so with >4-ish sems the NEFF fails codegen ("Too many sync wait
    commands"). Split the waits across single-wait sequencer nops instead.
    """
    nc = self.nc
    gc = tick_clock.global_clock
    ticks = list(gc)
    for idx, sem in self.sems.allocated().items():
        tick = ticks[idx]
        if tick <= 0:
            continue
        name = _IDX2PROC.get(idx, "")
        val = tick * (16 if name.startswith("DMA") else 1)
        nc.sync.nop().wait_op(sem, val, "sem-ge")
    nc.sync.drain()
    nc.all_engine_barrier()
    popped = nc._tile_sem_poison_stack.pop()
    assert popped is self._sem_poison
    nc.clear_and_free_semaphores(list(self.sems.allocated().values()))
    nc.all_engine_barrier()


tile.TileContext._drain_and_barrier = _split_drain_and_barrier

B, S, D = 8, 1024, 768
NCORES = 8
EPS = 1e-7
P = 128
KT = D // P  # 6 k-blocks of D
KP = KT // 2  # 3 k-pairs (DoubleRow contraction = 256)
NT = S // P  # 8 t-blocks of S
NQ = NT // 2  # 4 t-pairs
NJ = S // 512  # 2 column groups of 512
DVP = 784  # padded V row: 768 V cols + 1 mask col + 15 pad (16B-aligned pairs)
SA = 64.0  # scale on A for the QK matmul (psum = SA^2 * QK)
SF = 16.0  # scale on f = expm1(QK) for fp8
SV = 64.0  # scale on V (and mask col) for the AV matmul
NWARM = 8  # PE warm-up dummy matmuls (cover the input-DMA wait)

F8 = mybir.dt.float8e4
F16 = mybir.dt.float16
F32 = mybir.dt.float32
NP_F8 = ml_dtypes.float8_e4m3  # TRN fp8e4 (max +-240), matches device
DR = mybir.MatmulPerfMode.DoubleRow

_cache = {}


def _build():
    nc = bass.Bass()
    atp = nc.declare_dram_parameter("atp", [P, KT * S], F8, isOutput=False)
    avp = nc.declare_dram_parameter("avp", [P, NT * DVP], F8, isOutput=False)
    cst = nc.declare_dram_parameter("cst", [P, 1], F32, isOutput=False)
    out = nc.declare_dram_parameter("out", [S, D], F16, isOutput=True)
    orv = nc.declare_dram_parameter("orv", [P, NT], F32, isOutput=True)

    with tile.TileContext(nc) as tc:
        with (
            tc.tile_pool(name="w", bufs=1) as wpool,
            tc.tile_pool(name="e", bufs=1) as epool,
            tc.tile_pool(name="esc", bufs=1) as escpool,
            tc.tile_pool(name="qkps", bufs=3, space="PSUM") as qkps,
            tc.tile_pool(name="avps", bufs=1, space="PSUM") as avps,
            tc.tile_pool(name="o", bufs=1) as opool,
            tc.tile_pool(name="s", bufs=1) as spool,
        ):
            # --- warm-up garbage tile (memset so it's defined, no NaNs) ---
            gt = wpool.tile([P, 512], F8, name="gt", tag="gt")
            nc.gpsimd.memset(gt[:, :], 0.0)

            # --- inputs ---
            # at8[p, k, s] = fp8(SA * A[s, 128*k + p]).  One DMA on the
            # scalar engine: its queue is free ~1.2us before sync's, and
            # chunked dma_starts serialize ~2.5us each on the DGE anyway.
            att = wpool.tile([P, KT, S], F8, name="att", tag="att")
            atv = atp.rearrange("p (k s) -> p k s", k=KT)
            nc.scalar.dma_start(att[:, 0:2, :], atv[:, 0:2, :])
            nc.sync.dma_start(att[:, 2:4, :], atv[:, 2:4, :])
            nc.gpsimd.dma_start(att[:, 4:6, :], atv[:, 4:6, :])
            # av8[p, t, d] = fp8(SV * Vm[128*t + p, d]), col 768 = SV*mask.
            # These ride the gpsimd SWDGE queue: only 8 HWDGE sem lanes
            # exist and the 3 input chunks + 4 output groups + orv use them.
            avt = wpool.tile([P, NT, DVP], F8, name="avt", tag="avt")
            nc.gpsimd.dma_start(
                avt[:, :, :], avp.rearrange("p (t d) -> p t d", t=NT)
            )
            ctile = wpool.tile([P, 1], F32, name="ct", tag="ct")
            nc.gpsimd.dma_start(ctile[:, :], cst[:, :])

            # --- AV psum (allocated early; doubles as warm-up target) ---
            pabuf = [
                avps.tile([P, 512], F32, tag=f"pa{x}", name=f"pa{x}")
                for x in range(3)
            ]
            pbbuf = [
                avps.tile([P, 257], F32, tag=f"pb{x}", name=f"pb{x}")
                for x in range(2)
            ]

            # --- PE warm-up: release the HAM clock gate during DMA wait ---
            for w in range(NWARM):
                nc.tensor.matmul(
                    pabuf[w % 2][:, :],
                    gt[:, 0:P],
                    gt[:, :],
                    start=True,
                    stop=True,
                    skip_group_check=True,
                )

            # Pre-touch ctile on the scalar engine: pays its DMA wait once
            # so the per-m rtmp activation needs no new waits.
            cttch = spool.tile([P, 1], F32, name="cttch", tag="cttch")
            nc.scalar.activation(
                cttch[:, :], ctile[:, :], mybir.ActivationFunctionType.Identity
            )
            cvec = spool.tile([P, 1], F32, name="cvec", tag="cvec")
            nc.vector.tensor_copy(cvec[:, :], ctile[:, :])

            # f tiles, pair layout for DoubleRow AV lhsT:
            # Ep[q][p, u, s] = f[(2q+u)*128 + p, s]
            Ep = [
                epool.tile([P, 2, S], F8, name=f"E{q}", tag=f"E{q}")
                for q in range(NQ)
            ]

            # --- QK^T, exp, f-conversion ---
            for i in range(NT):
                for j in range(NJ):
                    ps = qkps.tile([P, 512], F32, tag="qk", name=f"qk{i}_{j}")
                    for q in range(KP):
                        nc.tensor.matmul(
                            ps[:, :],
                            att[:, 2 * q : 2 * q + 2, i * P : (i + 1) * P],
                            att[:, 2 * q : 2 * q + 2, j * 512 : (j + 1) * 512],
                            start=(q == 0),
                            stop=(q == KP - 1),
                            perf_mode=DR,
                        )
                    esc = escpool.tile(
                        [P, 512], F16, tag=f"esc{i}_{j}", name=f"esc{i}_{j}"
                    )
                    nc.scalar.activation(
                        esc[:, :],
                        ps[:, :],
                        mybir.ActivationFunctionType.Exp,
                        scale=1.0 / (SA * SA),
                    )
                    nc.vector.tensor_scalar(
                        out=Ep[i // 2][:, i % 2, j * 512 : (j + 1) * 512],
                        in0=esc[:, :],
                        scalar1=1.0,
                        scalar2=SF,
                        op0=mybir.AluOpType.subtract,
                        op1=mybir.AluOpType.mult,
                    )

            # --- AV: U = f @ [V | m], rows scaled by 1/(1024*(r+eps)) ---
            OG = [2, 2, 2, 1, 1]  # m-blocks per output DMA group
            ots = [
                opool.tile([P, g, D], F16, name=f"ot{i}", tag=f"ot{i}")
                for i, g in enumerate(OG)
            ]
            m2g = []  # m -> (group, slot)
            for gi, g in enumerate(OG):
                for s in range(g):
                    m2g.append((gi, s))
            obase = [0, 2, 4, 6, 7]

            pam7 = qkps.tile([P, 512], F32, tag="qk", name="pam7")
            pbm7 = qkps.tile([P, 512], F32, tag="qk", name="pbm7")

            def av_mms(m, q):
                lhsT = Ep[q][:, :, m * P : (m + 1) * P]
                pa_t = pam7 if m == NT - 1 else pabuf[m % 3]
                pb_t = pbm7 if m == NT - 1 else pbbuf[m % 2]
                nc.tensor.matmul(
                    pa_t[:, :],
                    lhsT,
                    avt[:, 2 * q : 2 * q + 2, 0:512],
                    start=(q == 0),
                    stop=(q == NQ - 1),
                    perf_mode=DR,
                    skip_group_check=True,
                )
                nc.tensor.matmul(
                    pb_t[:, 0:257],
                    lhsT,
                    avt[:, 2 * q : 2 * q + 2, 512 : 512 + 257],
                    start=(q == 0),
                    stop=(q == NQ - 1),
                    perf_mode=DR,
                    skip_group_check=True,
                )

            rtmps = {}

            orvt = spool.tile([P, NT - 1], F32, name="orvt", tag="orvt")
            orvB = spool.tile([P, 1], F32, name="orvB", tag="orvB")

            def av_tail(m):
                # Tail on the scalar engine (two raw psum->SBUF fp16 copies,
                # one PE wait each; rtmp = Identity(pb_col + ctile) via the
                # activation bias AP).  m7 runs on the DVE instead, reading
                # fresh qk-pool banks, so the final block's evacuation does
                # not serialize behind the scalar queue.  Division and
                # colsum land on the host.
                gi, slot = m2g[m]
                ot = ots[gi][:, slot, :]
                if m == NT - 1:
                    pa, pb = pam7, pbm7
                    nc.vector.tensor_copy(ot[:, 0:512], pa[:, :])
                    nc.sync.dma_start(
                        out[m * P : (m + 1) * P, 0:512], ot[:, 0:512]
                    )
                    nc.vector.tensor_copy(ot[:, 512:D], pb[:, 0:256])
                    t1 = spool.tile([P, 1], F32, tag="t1m7", name="t1m7")
                    nc.vector.tensor_copy(t1[:, :], pb[:, 256:257])
                    nc.vector.tensor_scalar_add(
                        orvB[:, 0:1], t1[:, :], cvec[:, :]
                    )
                    nc.sync.dma_start(
                        out[m * P : (m + 1) * P, 512:D], ot[:, 512:D]
                    )
                    return
                pa, pb = pabuf[m % 3], pbbuf[m % 2]
                nc.scalar.activation(
                    ot[:, 0:512], pa[:, :], mybir.ActivationFunctionType.Copy
                )
                nc.scalar.activation(
                    ot[:, 512:D], pb[:, 0:256], mybir.ActivationFunctionType.Copy
                )
                nc.scalar.activation(
                    orvt[:, m : m + 1],
                    pb[:, 256:257],
                    mybir.ActivationFunctionType.Identity,
                    bias=ctile[:, :],
                )
                if slot == OG[gi] - 1:
                    nc.sync.dma_start(
                        out[obase[gi] * P : (obase[gi] + OG[gi]) * P, :].rearrange(
                            "(b p) d -> p b d", p=P
                        ),
                        ots[gi][:, :, :],
                    )

            # Plain per-m order: each m's q0-q2 matmuls (~2.6us) cover the
            # exp+f-conversion latency of the last QK tile that q3 needs.
            for m in range(NT):
                for q in range(NQ):
                    av_mms(m, q)
                av_tail(m)
            nc.gpsimd.dma_start(orv[:, 0 : NT - 1], orvt[:, :])
            nc.gpsimd.dma_start(orv[:, NT - 1 : NT], orvB[:, :])

    return nc


def _get_nc():
    if "nc" not in _cache:
        _cache["nc"] = _build()
    return _cache["nc"]


def kernel(input_a, input_mask, _trace=False, **_kw):
    A = np.asarray(input_a, dtype=np.float32)  # [B, S, D]
    M = np.asarray(input_mask)  # [B, S] int32

    in_maps = []
    host = []
    for b in range(B):
        a = A[b]
        mf = M[b].astype(np.float32)
        vm = a * mf[:, None]

        a8 = (a * SA).astype(NP_F8)  # [S, D]
        # atp[p, k*1024+s] = a8[s, 128*k+p]
        atp = np.ascontiguousarray(
            a8.T.reshape(KT, P, S).transpose(1, 0, 2).reshape(P, KT * S)
        )
        v8 = np.zeros((NT, P, DVP), NP_F8)
        v8[:, :, :D] = (vm * SV).astype(NP_F8).reshape(NT, P, D)
        v8[:, :, D] = (mf * SV).astype(NP_F8).reshape(NT, P)
        avp = np.ascontiguousarray(
            v8.transpose(1, 0, 2).reshape(P, NT * DVP)
        )
        summ = float(mf.sum())
        cstv = np.full((P, 1), SF * SV * (summ + EPS), np.float32)
        in_maps.append({"atp": atp, "avp": avp, "cst": cstv})
        host.append((mf, vm.sum(axis=0)))

    nc = _get_nc()
    res = bass_utils.run_bass_kernel_spmd(
        nc, in_maps, core_ids=list(range(NCORES)), trace=_trace
    )
    outp = np.empty((B, S, D), np.float32)
    for b in range(B):
        mf, colsum = host[b]
        raw = res.results[b]["out"].astype(np.float32)  # 1024*(f@Vm)
        r1024 = res.results[b]["orv"].astype(np.float32).T.reshape(S)
        rfac = 1.0 / r1024  # = 1/(1024*(r+eps))
        outp[b] = rfac[:, None] * (raw + SF * SV * colsum[None, :]) * mf[:, None]
    if _trace:
        kernel.last_results = res
    return outp


# revision 34
# speedup vs baseline: 1.0798x; 1.0798x over previous
"""Self-attention (IntraSelfAttention) kernel for Trainium2, 8-core data
parallel, fp8e4m3 DoubleRow matmuls.

Math (per batch element b, on one core), with E = exp(A @ A.T):
    out = (E @ Vm) / (E @ m + eps) * m_row,   Vm = A * m[:,None]

Decompose E = 1 + f, f = expm1(QK).  Then
    E @ Vm = colsum(Vm) + f @ Vm        (colsum added on host, rank-1)
    E @ m  = sum(m) + f @ m             (sum(m) DMA'd as a constant)
f has zero mean +- 0.07 off-diagonal, so it survives fp8e4m3 quantization
(x16) where E itself (~1.0, fp8 ulp 0.125) would not.  Both matmuls run
in fp8 DoubleRow mode (2 contraction rows/cycle):
    QK:  psum = sum_k at8[:,2k:2k+2,i] .T@ at8[:,2k:2k+2,j]   (= 4096*QK)
    exp: scalar activation Exp, scale=1/4096 -> E fp16 scratch
    f:   gpsimd (E - 1)*16 -> fp8 tiles Epair[q][p, u, s]
    AV:  pa = f8 @ v8[0:512], pb = f8 @ v8[512:769] (col 768 = 64*mask)
    out = pa * rinv (fp16), rinv = 1/(pb[:,256] + 1024*(summ+eps))
Host: out_full = (out + 1024*rinv * colsum(Vm)) * mask_row.

Symmetry of f means QK's output tiles are directly usable as AV lhsT.
Warm-up dummy matmuls run while the input DMA streams so the PE HAM
clock-gate is released (2.4 GHz) by the time real matmuls start.
"""

import os
import numpy as np
import ml_dtypes

try:
    import concourse.bass as bass
except ImportError:
    import sys

    sys.path.insert(0, "/opt/trn_rl_repo")
    import concourse.bass as bass

import concourse.mybir as mybir
import concourse.tile as tile
from concourse import bass_utils
from concourse.tile_sem_assignment import PROC_NAME_TO_IDX

_IDX2PROC = {v: k for k, v in PROC_NAME_TO_IDX.items()}


def _split_drain_and_barrier(self, tick_clock, wait_clock):
    """Replacement for TileContext._drain_and_barrier.

    The stock version attaches every outstanding semaphore wait to the single
    kernel-tail Drain instruction; walrus's per-instruction sync-wait capacity
    is tiny, so with >4-ish sems the NEFF fails codegen ("Too many sync wait
    commands"). Split the waits across single-wait sequencer nops instead.
    """
    nc = self.nc
    gc = tick_clock.global_clock
    ticks = list(gc)
    for idx, sem in self.sems.allocated().items():
        tick = ticks[idx]
        if tick <= 0:
            continue
        name = _IDX2PROC.get(idx, "")
        val = tick * (16 if name.startswith("DMA") else 1)
        nc.sync.nop().wait_op(sem, val, "sem-ge")
    nc.sync.drain()
    nc.all_engine_barrier()
    popped = nc._tile_sem_poison_stack.pop()
    assert popped is self._sem_poison
    nc.clear_and_free_semaphores(list(self.sems.allocated().values()))
    nc.all_engine_barrier()


tile.TileContext._drain_and_barrier = _split_drain_and_barrier

B, S, D = 8, 1024, 768
NCORES = 8
EPS = 1e-7
P = 128
KT = D // P  # 6 k-blocks of D
KP = KT // 2  # 3 k-pairs (DoubleRow contraction = 256)
NT = S // P  # 8 t-blocks of S
NQ = NT // 2  # 4 t-pairs
NJ = S // 512  # 2 column groups of 512
DVP = 784  # padded V row: 768 V cols + 1 mask col + 15 pad (16B-aligned pairs)
SA = 64.0  # scale on A for the QK matmul (psum = SA^2 * QK)
SF = 16.0  # scale on f = expm1(QK) for fp8
SV = 64.0  # scale on V (and mask col) for the AV matmul
NWARM = 8  # PE warm-up dummy matmuls (cover the input-DMA wait)

F8 = mybir.dt.float8e4
F16 = mybir.dt.float16
F32 = mybir.dt.float32
NP_F8 = ml_dtypes.float8_e4m3  # TRN fp8e4 (max +-240), matches device
DR = mybir.MatmulPerfMode.DoubleRow

_cache = {}


def _build():
    nc = bass.Bass()
    atp = nc.declare_dram_parameter("atp", [P, KT * S], F8, isOutput=False)
    avp = nc.declare_dram_parameter("avp", [P, NT * DVP], F8, isOutput=False)
    cst = nc.declare_dram_parameter("cst", [P, 1], F32, isOutput=False)
    out = nc.declare_dram_parameter("out", [S, D], F16, isOutput=True)
    orv = nc.declare_dram_parameter("orv", [P, NT], F32, isOutput=True)

    with tile.TileContext(nc) as tc:
        with (
            tc.tile_pool(name="w", bufs=1) as wpool,
            tc.tile_pool(name="e", bufs=1) as epool,
            tc.tile_pool(name="esc", bufs=1) as escpool,
            tc.tile_pool(name="qkps", bufs=3, space="PSUM") as qkps,
            tc.tile_pool(name="avps", bufs=1, space="PSUM") as avps,
            tc.tile_pool(name="o", bufs=1) as opool,
            tc.tile_pool(name="s", bufs=1) as spool,
        ):
            # --- warm-up garbage tile (memset so it's defined, no NaNs) ---
            gt = wpool.tile([P, 512], F8, name="gt", tag="gt")
            nc.gpsimd.memset(gt[:, :], 0.0)

            # --- inputs ---
            # at8[p, k, s] = fp8(SA * A[s, 128*k + p]).  One DMA on the
            # scalar engine: its queue is free ~1.2us before sync's, and
            # chunked dma_starts serialize ~2.5us each on the DGE anyway.
            att = wpool.tile([P, KT, S], F8, name="att", tag="att")
            atv = atp.rearrange("p (k s) -> p k s", k=KT)
            nc.scalar.dma_start(att[:, 0:2, :], atv[:, 0:2, :])
            nc.sync.dma_start(att[:, 2:4, :], atv[:, 2:4, :])
            nc.gpsimd.dma_start(att[:, 4:6, :], atv[:, 4:6, :])
            # av8[p, t, d] = fp8(SV * Vm[128*t + p, d]), col 768 = SV*mask.
            # These ride the gpsimd SWDGE queue: only 8 HWDGE sem lanes
            # exist and the 3 input chunks + 4 output groups + orv use them.
            avt = wpool.tile([P, NT, DVP], F8, name="avt", tag="avt")
            nc.gpsimd.dma_start(
                avt[:, :, :], avp.rearrange("p (t d) -> p t d", t=NT)
            )
            ctile = wpool.tile([P, 1], F32, name="ct", tag="ct")
            nc.gpsimd.dma_start(ctile[:, :], cst[:, :])

            # --- AV psum (allocated early; doubles as warm-up target) ---
            pabuf = [
                avps.tile([P, 512], F32, tag=f"pa{x}", name=f"pa{x}")
                for x in range(3)
            ]
            pbbuf = [
                avps.tile([P, 257], F32, tag=f"pb{x}", name=f"pb{x}")
                for x in range(2)
            ]

            # --- PE warm-up: release the HAM clock gate during DMA wait ---
            for w in range(NWARM):
                nc.tensor.matmul(
                    pabuf[w % 2][:, :],
                    gt[:, 0:P],
                    gt[:, :],
                    start=True,
                    stop=True,
                    skip_group_check=True,
                )

            # Pre-touch ctile on the scalar engine: pays its DMA wait once
            # so the per-m rtmp activation needs no new waits.
            cttch = spool.tile([P, 1], F32, name="cttch", tag="cttch")
            nc.scalar.activation(
                cttch[:, :], ctile[:, :], mybir.ActivationFunctionType.Identity
            )


            # f tiles, pair layout for DoubleRow AV lhsT:
            # Ep[q][p, u, s] = f[(2q+u)*128 + p, s]
            Ep = [
                epool.tile([P, 2, S], F8, name=f"E{q}", tag=f"E{q}")
                for q in range(NQ)
            ]

            # --- QK^T, exp, f-conversion ---
            for i in range(NT):
                for j in range(NJ):
                    ps = qkps.tile([P, 512], F32, tag="qk", name=f"qk{i}_{j}")
                    for q in range(KP):
                        nc.tensor.matmul(
                            ps[:, :],
                            att[:, 2 * q : 2 * q + 2, i * P : (i + 1) * P],
                            att[:, 2 * q : 2 * q + 2, j * 512 : (j + 1) * 512],
                            start=(q == 0),
                            stop=(q == KP - 1),
                            perf_mode=DR,
                        )
                    esc = escpool.tile(
                        [P, 512], F16, tag=f"esc{i}_{j}", name=f"esc{i}_{j}"
                    )
                    nc.scalar.activation(
                        esc[:, :],
                        ps[:, :],
                        mybir.ActivationFunctionType.Exp,
                        scale=1.0 / (SA * SA),
                    )
                    nc.vector.tensor_scalar(
                        out=Ep[i // 2][:, i % 2, j * 512 : (j + 1) * 512],
                        in0=esc[:, :],
                        scalar1=1.0,
                        scalar2=SF,
                        op0=mybir.AluOpType.subtract,
                        op1=mybir.AluOpType.mult,
                    )

            # --- AV: U = f @ [V | m], rows scaled by 1/(1024*(r+eps)) ---
            OG = [2, 2, 2, 1, 1]  # m-blocks per output DMA group
            ots = [
                opool.tile([P, g, D], F16, name=f"ot{i}", tag=f"ot{i}")
                for i, g in enumerate(OG)
            ]
            m2g = []  # m -> (group, slot)
            for gi, g in enumerate(OG):
                for s in range(g):
                    m2g.append((gi, s))
            obase = [0, 2, 4, 6, 7]

            def av_mms(m, q):
                lhsT = Ep[q][:, :, m * P : (m + 1) * P]
                nc.tensor.matmul(
                    pabuf[m % 3][:, :],
                    lhsT,
                    avt[:, 2 * q : 2 * q + 2, 0:512],
                    start=(q == 0),
                    stop=(q == NQ - 1),
                    perf_mode=DR,
                    skip_group_check=True,
                )
                nc.tensor.matmul(
                    pbbuf[m % 2][:, 0:257],
                    lhsT,
                    avt[:, 2 * q : 2 * q + 2, 512 : 512 + 257],
                    start=(q == 0),
                    stop=(q == NQ - 1),
                    perf_mode=DR,
                    skip_group_check=True,
                )

            rtmps = {}

            orvt = spool.tile([P, NT], F32, name="orvt", tag="orvt")

            def av_tail(m):
                # Entire tail on the scalar engine: two raw psum->SBUF fp16
                # copies (one PE wait each) and rtmp = Identity(pb_col+ctile)
                # via the activation bias AP.  The last block's first 512
                # columns ship while its pb copy still runs.  Division and
                # colsum land on the host.
                pa, pb = pabuf[m % 3], pbbuf[m % 2]
                gi, slot = m2g[m]
                ot = ots[gi][:, slot, :]
                nc.scalar.activation(
                    ot[:, 0:512], pa[:, :], mybir.ActivationFunctionType.Copy
                )
                if m == NT - 1:
                    nc.sync.dma_start(
                        out[m * P : (m + 1) * P, 0:512], ot[:, 0:512]
                    )
                nc.scalar.activation(
                    ot[:, 512:D], pb[:, 0:256], mybir.ActivationFunctionType.Copy
                )
                nc.scalar.activation(
                    orvt[:, m : m + 1],
                    pb[:, 256:257],
                    mybir.ActivationFunctionType.Identity,
                    bias=ctile[:, :],
                )
                if m == NT - 1:
                    nc.sync.dma_start(
                        out[m * P : (m + 1) * P, 512:D], ot[:, 512:D]
                    )
                elif slot == OG[gi] - 1:
                    nc.sync.dma_start(
                        out[obase[gi] * P : (obase[gi] + OG[gi]) * P, :].rearrange(
                            "(b p) d -> p b d", p=P
                        ),
                        ots[gi][:, :, :],
                    )

            # Plain per-m order: each m's q0-q2 matmuls (~2.6us) cover the
            # exp+f-conversion latency of the last QK tile that q3 needs.
            for m in range(NT):
                for q in range(NQ):
                    av_mms(m, q)
                av_tail(m)
            nc.gpsimd.dma_start(orv[:, :], orvt[:, :])

    return nc


def _get_nc():
    if "nc" not in _cache:
        _cache["nc"] = _build()
    return _cache["nc"]


def kernel(input_a, input_mask, _trace=False, **_kw):
    A = np.asarray(input_a, dtype=np.float32)  # [B, S, D]
    M = np.asarray(input_mask)  # [B, S] int32

    in_maps = []
    host = []
    for b in range(B):
        a = A[b]
        mf = M[b].astype(np.float32)
        vm = a * mf[:, None]

        a8 = (a * SA).astype(NP_F8)  # [S, D]
        # atp[p, k*1024+s] = a8[s, 128*k+p]
        atp = np.ascontiguousarray(
            a8.T.reshape(KT, P, S).transpose(1, 0, 2).reshape(P, KT * S)
        )
        v8 = np.zeros((NT, P, DVP), NP_F8)
        v8[:, :, :D] = (vm * SV).astype(NP_F8).reshape(NT, P, D)
        v8[:, :, D] = (mf * SV).astype(NP_F8).reshape(NT, P)
        avp = np.ascontiguousarray(
            v8.transpose(1, 0, 2).reshape(P, NT * DVP)
        )
        summ = float(mf.sum())
        cstv = np.full((P, 1), SF * SV * (summ + EPS), np.float32)
        in_maps.append({"atp": atp, "avp": avp, "cst": cstv})
        host.append((mf, vm.sum(axis=0)))

    nc = _get_nc()
    res = bass_utils.run_bass_kernel_spmd(
        nc, in_maps, core_ids=list(range(NCORES)), trace=_trace
    )
    outp = np.empty((B, S, D), np.float32)
    for b in range(B):
        mf, colsum = host[b]
        raw = res.results[b]["out"].astype(np.float32)  # 1024*(f@Vm)
        r1024 = res.results[b]["orv"].astype(np.float32).T.reshape(S)
        rfac = 1.0 / r1024  # = 1/(1024*(r+eps))
        outp[b] = rfac[:, None] * (raw + SF * SV * colsum[None, :]) * mf[:, None]
    if _trace:
        kernel.last_results = res
    return outp


# revision 37
# speedup vs baseline: 1.1437x; 1.0591x over previous
"""Self-attention (IntraSelfAttention) kernel for Trainium2, 8-core data
parallel, fp8e4m3 DoubleRow matmuls.

Math (per batch element b, on one core), with E = exp(A @ A.T):
    out = (E @ Vm) / (E @ m + eps) * m_row,   Vm = A * m[:,None]

Decompose E = 1 + f, f = expm1(QK).  Then
    E @ Vm = colsum(Vm) + f @ Vm        (colsum added on host, rank-1)
    E @ m  = sum(m) + f @ m             (sum(m) DMA'd as a constant)
f has zero mean +- 0.07 off-diagonal, so it survives fp8e4m3 quantization
(x16) where E itself (~1.0, fp8 ulp 0.125) would not.  Both matmuls run
in fp8 DoubleRow mode (2 contraction rows/cycle):
    QK:  psum = sum_k at8[:,2k:2k+2,i] .T@ at8[:,2k:2k+2,j]   (= 4096*QK)
    exp: scalar activation Exp, scale=1/4096 -> E fp16 scratch
    f:   gpsimd (E - 1)*16 -> fp8 tiles Epair[q][p, u, s]
    AV:  pa = f8 @ v8[0:512], pb = f8 @ v8[512:769] (col 768 = 64*mask)
    out = pa * rinv (fp16), rinv = 1/(pb[:,256] + 1024*(summ+eps))
Host: out_full = (out + 1024*rinv * colsum(Vm)) * mask_row.

Symmetry of f means QK's output tiles are directly usable as AV lhsT.
Warm-up dummy matmuls run while the input DMA streams so the PE HAM
clock-gate is released (2.4 GHz) by the time real matmuls start.
"""

import os
import numpy as np
import ml_dtypes

try:
    import concourse.bass as bass
except ImportError:
    import sys

    sys.path.insert(0, "/opt/trn_rl_repo")
    import concourse.bass as bass

import concourse.mybir as mybir
import concourse.tile as tile
from concourse import bass_utils
from concourse.tile_sem_assignment import PROC_NAME_TO_IDX

_IDX2PROC = {v: k for k, v in PROC_NAME_TO_IDX.items()}


def _split_drain_and_barrier(self, tick_clock, wait_clock):
    """Replacement for TileContext._drain_and_barrier.

    The stock version attaches every outstanding semaphore wait to the single
    kernel-tail Drain instruction; walrus's per-instruction sync-wait capacity
    is tiny, so with >4-ish sems the NEFF fails codegen ("Too many sync wait
    commands"). Split the waits across single-wait sequencer nops instead.
    """
    nc = self.nc
    gc = tick_clock.global_clock
    ticks = list(gc)
    for idx, sem in self.sems.allocated().items():
        tick = ticks[idx]
        if tick <= 0:
            continue
        name = _IDX2PROC.get(idx, "")
        val = tick * (16 if name.startswith("DMA") else 1)
        nc.sync.nop().wait_op(sem, val, "sem-ge")
    nc.sync.drain()
    nc.all_engine_barrier()
    popped = nc._tile_sem_poison_stack.pop()
    assert popped is self._sem_poison
    nc.clear_and_free_semaphores(list(self.sems.allocated().values()))
    nc.all_engine_barrier()


tile.TileContext._drain_and_barrier = _split_drain_and_barrier

B, S, D = 8, 1024, 768
NCORES = 8
EPS = 1e-7
P = 128
KT = D // P  # 6 k-blocks of D
KP = KT // 2  # 3 k-pairs (DoubleRow contraction = 256)
NT = S // P  # 8 t-blocks of S
NQ = NT // 2  # 4 t-pairs
NJ = S // 512  # 2 column groups of 512
DVP = 784  # padded V row: 768 V cols + 1 mask col + 15 pad (16B-aligned pairs)
SA = 64.0  # scale on A for the QK matmul (psum = SA^2 * QK)
SF = 16.0  # scale on f = expm1(QK) for fp8
SV = 64.0  # scale on V (and mask col) for the AV matmul
NWARM = 8  # PE warm-up dummy matmuls (cover the input-DMA wait)

F8 = mybir.dt.float8e4
F16 = mybir.dt.float16
F32 = mybir.dt.float32
NP_F8 = ml_dtypes.float8_e4m3  # TRN fp8e4 (max +-240), matches device
DR = mybir.MatmulPerfMode.DoubleRow

_cache = {}


def _build():
    nc = bass.Bass()
    atp = nc.declare_dram_parameter("atp", [P, KT * S], F8, isOutput=False)
    avp = nc.declare_dram_parameter("avp", [P, NT * DVP], F8, isOutput=False)
    cst = nc.declare_dram_parameter("cst", [P, 1], F32, isOutput=False)
    out = nc.declare_dram_parameter("out", [S, D], F16, isOutput=True)
    orv = nc.declare_dram_parameter("orv", [P, NT], F32, isOutput=True)

    with tile.TileContext(nc) as tc:
        with (
            tc.tile_pool(name="w", bufs=1) as wpool,
            tc.tile_pool(name="e", bufs=1) as epool,
            tc.tile_pool(name="esc", bufs=1) as escpool,
            tc.tile_pool(name="qkps", bufs=2, space="PSUM") as qkps,
            tc.tile_pool(name="avps", bufs=1, space="PSUM") as avps,
            tc.tile_pool(name="o", bufs=1) as opool,
            tc.tile_pool(name="s", bufs=1) as spool,
        ):
            # --- warm-up garbage tile (memset so it's defined, no NaNs) ---
            gt = wpool.tile([P, 512], F8, name="gt", tag="gt")
            nc.gpsimd.memset(gt[:, :], 0.0)

            # --- inputs ---
            # at8[p, k, s] = fp8(SA * A[s, 128*k + p]).  One DMA on the
            # scalar engine: its queue is free ~1.2us before sync's, and
            # chunked dma_starts serialize ~2.5us each on the DGE anyway.
            att = wpool.tile([P, KT, S], F8, name="att", tag="att")
            atv = atp.rearrange("p (k s) -> p k s", k=KT)
            nc.sync.dma_start(att[:, 0:2, :], atv[:, 0:2, :])
            nc.scalar.dma_start(att[:, 2:6, :], atv[:, 2:6, :])
            # av8[p, t, d] = fp8(SV * Vm[128*t + p, d]), col 768 = SV*mask.
            # These ride the gpsimd SWDGE queue: only 8 HWDGE sem lanes
            # exist and the 3 input chunks + 4 output groups + orv use them.
            avt = wpool.tile([P, NT, DVP], F8, name="avt", tag="avt")
            nc.gpsimd.dma_start(
                avt[:, :, :], avp.rearrange("p (t d) -> p t d", t=NT)
            )
            ctile = wpool.tile([P, 1], F32, name="ct", tag="ct")
            nc.gpsimd.dma_start(ctile[:, :], cst[:, :])

            # --- AV psum (allocated early; doubles as warm-up target) ---
            pabuf = [
                avps.tile([P, 512], F32, tag=f"pa{x}", name=f"pa{x}")
                for x in range(2)
            ]
            pam7 = avps.tile([P, 512], F32, tag="pam7", name="pam7")
            pbm7 = avps.tile([P, 257], F32, tag="pbm7", name="pbm7")
            pbbuf = [
                avps.tile([P, 257], F32, tag=f"pb{x}", name=f"pb{x}")
                for x in range(2)
            ]

            # --- PE warm-up: release the HAM clock gate during DMA wait ---
            for w in range(NWARM):
                nc.tensor.matmul(
                    pabuf[w % 2][:, :],
                    gt[:, 0:P],
                    gt[:, :],
                    start=True,
                    stop=True,
                    skip_group_check=True,
                )

            # Pre-touch ctile on the scalar engine: pays its DMA wait once
            # so the per-m rtmp activation needs no new waits.
            cttch = spool.tile([P, 1], F32, name="cttch", tag="cttch")
            nc.scalar.activation(
                cttch[:, :], ctile[:, :], mybir.ActivationFunctionType.Identity
            )
            cvec = spool.tile([P, 1], F32, name="cvec", tag="cvec")
            nc.vector.tensor_copy(cvec[:, :], ctile[:, :])


            # f tiles, pair layout for DoubleRow AV lhsT:
            # Ep[q][p, u, s] = f[(2q+u)*128 + p, s]
            Ep = [
                epool.tile([P, 2, S], F8, name=f"E{q}", tag=f"E{q}")
                for q in range(NQ)
            ]

            # --- QK^T, exp, f-conversion ---
            for i in range(NT):
                for j in range(NJ):
                    ps = qkps.tile([P, 512], F32, tag="qk", name=f"qk{i}_{j}")
                    for q in range(KP):
                        nc.tensor.matmul(
                            ps[:, :],
                            att[:, 2 * q : 2 * q + 2, i * P : (i + 1) * P],
                            att[:, 2 * q : 2 * q + 2, j * 512 : (j + 1) * 512],
                            start=(q == 0),
                            stop=(q == KP - 1),
                            perf_mode=DR,
                        )
                    esc = escpool.tile(
                        [P, 512], F16, tag=f"esc{i}_{j}", name=f"esc{i}_{j}"
                    )
                    nc.scalar.activation(
                        esc[:, :],
                        ps[:, :],
                        mybir.ActivationFunctionType.Exp,
                        scale=1.0 / (SA * SA),
                    )
                    nc.vector.tensor_scalar(
                        out=Ep[i // 2][:, i % 2, j * 512 : (j + 1) * 512],
                        in0=esc[:, :],
                        scalar1=1.0,
                        scalar2=SF,
                        op0=mybir.AluOpType.subtract,
                        op1=mybir.AluOpType.mult,
                    )

            # --- AV: U = f @ [V | m], rows scaled by 1/(1024*(r+eps)) ---
            OG = [2, 2, 2, 1, 1]  # m-blocks per output DMA group
            ots = [
                opool.tile([P, g, D], F16, name=f"ot{i}", tag=f"ot{i}")
                for i, g in enumerate(OG)
            ]
            m2g = []  # m -> (group, slot)
            for gi, g in enumerate(OG):
                for s in range(g):
                    m2g.append((gi, s))
            obase = [0, 2, 4, 6, 7]

            def av_mms(m, q):
                lhsT = Ep[q][:, :, m * P : (m + 1) * P]
                pa_t = pam7 if m == NT - 1 else pabuf[m % 2]
                pb_t = pbm7 if m == NT - 1 else pbbuf[m % 2]
                nc.tensor.matmul(
                    pa_t[:, :],
                    lhsT,
                    avt[:, 2 * q : 2 * q + 2, 0:512],
                    start=(q == 0),
                    stop=(q == NQ - 1),
                    perf_mode=DR,
                    skip_group_check=True,
                )
                nc.tensor.matmul(
                    pb_t[:, 0:257],
                    lhsT,
                    avt[:, 2 * q : 2 * q + 2, 512 : 512 + 257],
                    start=(q == 0),
                    stop=(q == NQ - 1),
                    perf_mode=DR,
                    skip_group_check=True,
                )

            rtmps = {}

            orvt = spool.tile([P, NT - 1], F32, name="orvt", tag="orvt")

            orvB = spool.tile([P, 1], F32, name="orvB", tag="orvB")

            def av_tail(m):
                # Tail on the scalar engine: raw psum->SBUF fp16 copies (one
                # PE wait each) and rtmp = Identity(pb_col + ctile) via the
                # activation bias AP.  m7 runs entirely on the DVE from two
                # dedicated fresh banks, so the last block's evacuation and
                # output DMAs overlap the scalar queue's m6 tail.  Division
                # and colsum land on the host.
                gi, slot = m2g[m]
                ot = ots[gi][:, slot, :]
                if m == NT - 1:
                    nc.vector.tensor_copy(ot[:, 0:512], pam7[:, :])
                    nc.sync.dma_start(
                        out[m * P : (m + 1) * P, 0:512], ot[:, 0:512]
                    )
                    nc.vector.tensor_copy(ot[:, 512:D], pbm7[:, 0:256])
                    t1 = spool.tile([P, 1], F32, tag="t1m7", name="t1m7")
                    nc.vector.tensor_copy(t1[:, :], pbm7[:, 256:257])
                    nc.vector.tensor_scalar_add(orvB[:, 0:1], t1[:, :], cvec[:, :])
                    nc.scalar.dma_start(
                        out[m * P : (m + 1) * P, 512:D], ot[:, 512:D]
                    )
                    return
                pa, pb = pabuf[m % 2], pbbuf[m % 2]
                nc.scalar.activation(
                    ot[:, 0:512], pa[:, :], mybir.ActivationFunctionType.Copy
                )
                nc.scalar.activation(
                    ot[:, 512:D], pb[:, 0:256], mybir.ActivationFunctionType.Copy
                )
                nc.scalar.activation(
                    orvt[:, m : m + 1],
                    pb[:, 256:257],
                    mybir.ActivationFunctionType.Identity,
                    bias=ctile[:, :],
                )
                if slot == OG[gi] - 1:
                    nc.sync.dma_start(
                        out[obase[gi] * P : (obase[gi] + OG[gi]) * P, :].rearrange(
                            "(b p) d -> p b d", p=P
                        ),
                        ots[gi][:, :, :],
                    )

            # Plain per-m order: each m's q0-q2 matmuls (~2.6us) cover the
            # exp+f-conversion latency of the last QK tile that q3 needs.
            for m in range(NT):
                for q in range(NQ):
                    av_mms(m, q)
                av_tail(m)
            nc.gpsimd.dma_start(orv[:, 0 : NT - 1], orvt[:, :])
            nc.gpsimd.dma_start(orv[:, NT - 1 : NT], orvB[:, :])

    return nc


def _get_nc():
    if "nc" not in _cache:
        _cache["nc"] = _build()
    return _cache["nc"]


def kernel(input_a, input_mask, _trace=False, **_kw):
    A = np.asarray(input_a, dtype=np.float32)  # [B, S, D]
    M = np.asarray(input_mask)  # [B, S] int32

    in_maps = []
    host = []
    for b in range(B):
        a = A[b]
        mf = M[b].astype(np.float32)
        vm = a * mf[:, None]

        a8 = (a * SA).astype(NP_F8)  # [S, D]
        # atp[p, k*1024+s] = a8[s, 128*k+p]
        atp = np.ascontiguousarray(
            a8.T.reshape(KT, P, S).transpose(1, 0, 2).reshape(P, KT * S)
        )
        v8 = np.zeros((NT, P, DVP), NP_F8)
        v8[:, :, :D] = (vm * SV).astype(NP_F8).reshape(NT, P, D)
        v8[:, :, D] = (mf * SV).astype(NP_F8).reshape(NT, P)
        avp = np.ascontiguousarray(
            v8.transpose(1, 0, 2).reshape(P, NT * DVP)
        )
        summ = float(mf.sum())
        cstv = np.full((P, 1), SF * SV * (summ + EPS), np.float32)
        in_maps.append({"atp": atp, "avp": avp, "cst": cstv})
        host.append((mf, vm.sum(axis=0)))

    nc = _get_nc()
    res = bass_utils.run_bass_kernel_spmd(
        nc, in_maps, core_ids=list(range(NCORES)), trace=_trace
    )
    outp = np.empty((B, S, D), np.float32)
    for b in range(B):
        mf, colsum = host[b]
        raw = res.results[b]["out"].astype(np.float32)  # 1024*(f@Vm)
        r1024 = res.results[b]["orv"].astype(np.float32).T.reshape(S)
        rfac = 1.0 / r1024  # = 1/(1024*(r+eps))
        outp[b] = rfac[:, None] * (raw + SF * SV * colsum[None, :]) * mf[:, None]
    if _trace:
        kernel.last_results = res
    return outp
